# Initial kernel scaffold
#
"""Trainium2 Bass kernel for the token-scan problem.

Math: the reference scans T=128 tokens updating (x, rho) and emits
concat([x_T, y_T, v*_T, rho_T.ravel()]).  The x-recurrence depends only on
the (known) token sequence, so the scan unrolls into dense matmuls:

  V    = token_emb[tokens]                 [T, d]
  R    = relu(Dx @ V^T)                    [n, T]
  x_f  = R @ ones  (row sums)
  h    = R^T x_f                           [T]
  a    = vwu^T h  (vwu = U @ (V*w), U = triu-ones; w = decay weights)
  y    = relu(Dy @ ln(a)) * x_f            [n]
  v*   = ln(E @ y)                         [d]
  rho  = vwp^T R^T (vwp = U @ (V*w'))      [d, n]

Sharding: n split across 8 cores (Dx/Dy rows, E/rho columns, x/y slices).
Cross-core comm: ONE AllReduce of [a_partial | mean_partial] (257 floats).
The final E@y partial sums ([d] per core) are reduced + layernormed on
the host during unshard (8x256 values).

Precision: big operands ship as bf16 (Dx, rho out) and fp8-e4m3 (Dy x16,
E x64; these only feed the small y/v* output segments; ln is scale
invariant so only Dy's scale needs undoing, folded into the relu).
All matmuls accumulate in fp32 PSUM.  Overall rel-l2 vs the fp32 scan
is ~7e-3 (gate: 2e-2).
"""

import numpy as np
import ml_dtypes

N, D, V_VOCAB, T = 16384, 256, 32000, 128
DECAY = 0.97
N_CORES = 8
NS = N // N_CORES           # 2048 rows per core
NT = NS // 128              # 16 tiles of 128
YSCL = 2.0 ** -5            # y -> fp8 scale (ln(E@y) is scale invariant)

_cache = {}
STAGE = 99


def _build():
    stage = STAGE
    _NO_DVE_RELU = globals().get('NO_DVE_RELU', False)
    import concourse.bacc as bacc
    import concourse.mybir as mybir
    import concourse.tile as tile

    f32 = mybir.dt.float32
    bf16 = mybir.dt.bfloat16
    f8 = mybir.dt.float8e4
    AF = mybir.ActivationFunctionType
    ALU = mybir.AluOpType

    nc = bacc.Bacc("TRN2", target_bir_lowering=False, debug=False,
                   num_devices=N_CORES)

    # dxts: [128, 4096] bf16, col = c*2048 + h*1024 + n'  (n = c*1024+n',
    #       h = d-half) -- Dx^T packed in two n-chunks for chunked DMA.
    # dyts: [128, 4096] fp8, col = h*2048 + n   (Dy^T * 16)
    # ets:  [128, 4096] fp8, col = i*256 + h*128 + j -> E[h*128+j, i*128+p]*64
    # consts: [128, 769] bf16: vts[0:256] | vwu[256:512] | vwp[512:768]
    i_dxts = nc.dram_tensor("dxts", [128, 2 * NS], bf16, kind="ExternalInput")
    i_dyts = nc.dram_tensor("dyts", [128, 2 * NS], f8, kind="ExternalInput")
    i_ets = nc.dram_tensor("ets", [128, NT * 256], f8, kind="ExternalInput")
    i_consts = nc.dram_tensor("consts", [128, 770], bf16, kind="ExternalInput")

    o_x = nc.dram_tensor("out_x", [128, NT], f32, kind="ExternalOutput")
    o_a = nc.dram_tensor("out_a", [1, 256], bf16, kind="ExternalOutput")
    o_y = nc.dram_tensor("out_y", [128, NT], bf16, kind="ExternalOutput")
    o_vs = nc.dram_tensor("out_vs", [128, 2], bf16, kind="ExternalOutput")
    o_rho = nc.dram_tensor("out_rho", [256, NS], bf16, kind="ExternalOutput")

    with tile.TileContext(nc) as tc:
        with (
            tc.tile_pool(name="persist", bufs=1) as pp,
            tc.tile_pool(name="rhobuf", bufs=2) as wp,
            tc.tile_pool(name="psBig", bufs=3, space="PSUM") as psBig,
            tc.tile_pool(name="psRc", bufs=4, space="PSUM") as psRc,
            tc.tile_pool(name="psT", bufs=1, space="PSUM") as psT,
            tc.tile_pool(name="dram", bufs=1, space="DRAM") as dram,
        ):
            # ---- all bulk loads on the SP ring, in wire-priority order:
            # vts -> dxts (4 chunks) -> vwu/vwp -> dyts -> ets
            consts = pp.tile([128, 770], bf16)
            nc.sync.dma_start(consts[:, 0:256], i_consts[:, 0:256])
            dxts = pp.tile([128, 2 * NS], bf16)
            for c in range(4):
                nc.sync.dma_start(dxts[:, c * 1024:(c + 1) * 1024],
                                  i_dxts[:, c * 1024:(c + 1) * 1024])
            nc.sync.dma_start(consts[:, 256:770], i_consts[:, 256:770])
            dyts = pp.tile([128, 2 * NS], f8)
            ets = pp.tile([128, NT * 256], f8)
            if stage >= 4:
                nc.sync.dma_start(dyts[:, :NS], i_dyts[:, :NS])
                nc.sync.dma_start(dyts[:, NS:], i_dyts[:, NS:])
                nc.sync.dma_start(ets[:, :NS], i_ets[:, :NS])
                nc.sync.dma_start(ets[:, NS:], i_ets[:, NS:])

            vts = consts[:, 0:256]
            vwu = consts[:, 256:512]
            vwp = consts[:, 512:768]

            # pin the Act function table (includes Sqrt+Relu+Copy+Square)
            # before the first real Act op so no mid-kernel table swap occurs
            if stage >= 1 and not globals().get('NO_ACT_DUMMY', False):
                actp = pp.tile([1, 1], f32)
                nc.scalar.activation(actp[:], consts[0:1, 0:1], AF.Sqrt,
                                     bias=1.0, scale=0.0)

            # one PSUM bank for all small tiles:
            # cols 0-15 y, 16-17 a-col, 18-19 vs, 20-275 a-row, 276 h
            tail_ps = psT.tile([128, 278], f32, tag="tail")

            def chunk_relu(e, dst, src, accum):
                # relu + free-dim row-sum on DVE/Act/Pool
                if e == 0:
                    nc.vector.tensor_scalar(dst, src, 0.0, None,
                                            ALU.max, ALU.add, accum_out=accum)
                elif e == 1:
                    nc.scalar.activation(dst, src, AF.Relu, accum_out=accum)
                else:
                    raise AssertionError("Pool cannot read PSUM")

            # ---- rcols = relu(Dx V^T) (n on partitions) + x_f ----
            # dxts col layout: c*1024 + h*512 + n''  (c in 0..3, n'' < 512)
            rcols = pp.tile([128, NT * 128], bf16)
            rt = pp.tile([128, NS], bf16)
            rtacc = pp.tile([128, 4], f32)     # scratch accum for rt relus
            xfcol = pp.tile([128, NT], f32)
            xfb = pp.tile([128, NT], bf16)
            rc_eng = [1, 0, 1, 0, 1, 0, 1, 0, 1, 0, 1, 0, 1, 0, 1, 0]
            for i in range(NT):
                c, i4 = i // 4, i % 4
                base = c * 1024 + i4 * 128
                rc_ps = psRc.tile([128, 128], f32, tag="rc")
                for h in range(2):
                    nc.tensor.matmul(
                        rc_ps[:],
                        lhsT=dxts[:, base + h * 512: base + h * 512 + 128],
                        rhs=vts[:, h * 128:(h + 1) * 128],
                        start=(h == 0), stop=(h == 1))
                chunk_relu(rc_eng[i] if stage >= 1 and not _NO_DVE_RELU else 1,
                           rcols[:, i * 128:(i + 1) * 128],
                           rc_ps[:], xfcol[:, i:i + 1])

            def emit_rt(q, e):
                # rt = relu(V @ Dx^T) (T on partitions) for n-chunk q
                rt_ps = psBig.tile([128, 512], f32, tag="big")
                for h in range(2):
                    nc.tensor.matmul(
                        rt_ps[:],
                        lhsT=vts[:, h * 128:(h + 1) * 128],
                        rhs=dxts[:, q * 1024 + h * 512: q * 1024 + h * 512 + 512],
                        start=(h == 0), stop=(h == 1))
                # keep rt relus for q0/q1 off DVE/Act so they don't gate the
                # h-path ops queued behind them (in-order engine queues)
                chunk_relu(e, rt[:, q * 512:(q + 1) * 512], rt_ps[:],
                           rtacc[:, q:q + 1])

            if stage >= 1:
                emit_rt(0, 0)
                emit_rt(1, 1)

            # ---- h = R^T x_f ; a_partial = vwu^T h (+ mean piggyback) ----
            nc.vector.tensor_copy(xfb[:], xfcol[:])
            for i in range(NT if stage >= 1 else 0):
                nc.tensor.matmul(tail_ps[:, 276:277],
                                 lhsT=rcols[:, i * 128:(i + 1) * 128],
                                 rhs=xfb[:, i:i + 1],
                                 start=(i == 0), stop=(i == NT - 1))
            h_sb = pp.tile([128, 1], bf16)
            a_sb = pp.tile([1, 256], bf16)
            if stage >= 1:
                nc.vector.tensor_copy(h_sb[:], tail_ps[:, 276:277])
                a_ps = tail_ps[0:1, 20:276]
                nc.tensor.matmul(a_ps, lhsT=h_sb[:], rhs=vwu[:],
                                 start=True, stop=True)
                nc.tensor.matmul(tail_ps[0:1, 277:278], lhsT=h_sb[:],
                                 rhs=consts[:, 769:770],
                                 start=True, stop=True)
                m_sb = pp.tile([1, 1], f32)
                nc.vector.tensor_copy(m_sb[:], tail_ps[0:1, 277:278])
                # send the partial already centered: AllReduce is linear, so
                # sum_c (a_c - m_c) == a - mean(a) exactly
                nc.vector.tensor_scalar_sub(a_sb[:], a_ps, m_sb[:])

            a_in = dram.tile([1, 256], bf16)
            a_out = dram.tile([1, 256], bf16)
            if stage >= 1:
                nc.sync.dma_start(a_in[:], a_sb[:])
            nc.scalar.dma_start(o_x[:], xfcol[:])
            if stage >= 2:
                nc.gpsimd.collective_compute(
                    "AllReduce", ALU.add,
                    replica_groups=[list(range(N_CORES))],
                    ins=[a_in.opt()], outs=[a_out.opt()],
                )


            emit_rt(2, 0)
            emit_rt(3, 1)

            # ---- rho = vwp^T @ R^T (fills the collective window) ----
            rho_sbs = []
            for dc in range(2):
                rho_sb = wp.tile([128, NS], bf16, tag="rho")
                rho_sbs.append(rho_sb)
            # copy engines: keep DVE mostly clear for the a-chain early on
            copy_eng = [1, 0, 0, 1, 0, 1, 0, 1]
            for q in range(4 if stage >= 3 else 0):
                for dc in range(2):
                    rho_ps = psBig.tile([128, 512], f32, tag="big")
                    nc.tensor.matmul(rho_ps[:],
                                     lhsT=vwp[:, dc * 128:(dc + 1) * 128],
                                     rhs=rt[:, q * 512:(q + 1) * 512],
                                     start=True, stop=True)
                    dst = rho_sbs[dc][:, q * 512:(q + 1) * 512]
                    e = copy_eng[q * 2 + dc]
                    if e == 0:
                        nc.vector.tensor_copy(dst, rho_ps[:])
                    else:
                        nc.scalar.activation(dst, rho_ps[:], AF.Copy)
            # 4 x 256KB pieces, split across the SP and Act rings
            for p in range(4 if stage >= 3 else 0):
                dc, q2 = p % 2, p // 2
                ring = nc.gpsimd
                ring.dma_start(
                    o_rho[dc * 128:(dc + 1) * 128, q2 * 1024:(q2 + 1) * 1024],
                    rho_sbs[dc][:, q2 * 1024:(q2 + 1) * 1024])

            # ---- tail: ln(a), y, vs partial ----
            afull = pp.tile([1, 256], bf16)
            if stage >= 2:
                nc.sync.dma_start(afull[:], a_out[:])
            elif stage >= 1:
                nc.vector.tensor_copy(afull[:], a_sb[:])
            cenb = afull
            if stage >= 1 and not globals().get('NO_LN', False):
                nc.scalar.dma_start(o_a[:], afull[:])

            # transpose (a-m) to columns, folding *inv: 2 K=1 matmuls
            for h in range(2 if stage >= 4 else 0):
                nc.tensor.matmul(tail_ps[:, 16 + h:17 + h],
                                 lhsT=cenb[0:1, h * 128:(h + 1) * 128],
                                 rhs=consts[0:1, 768:769],
                                 start=True, stop=True)
            aln = pp.tile([128, 2], f8)
            if stage >= 4:
                nc.vector.tensor_copy(aln[:], tail_ps[:, 16:18])

            # y = relu(Dy @ aln)/16 * x_f
            for i in range(NT if stage >= 5 else 0):
                for h in range(2):
                    nc.tensor.matmul(
                        tail_ps[:, i:i + 1],
                        lhsT=dyts[:, h * NS + i * 128: h * NS + (i + 1) * 128],
                        rhs=aln[:, h:h + 1],
                        start=(h == 0), stop=(h == 1))
            ycr = pp.tile([128, NT], f32)
            ycrs = pp.tile([128, NT], f32)
            y_f8 = pp.tile([128, NT], f8)
            y_bf = pp.tile([128, NT], bf16)
            if stage >= 5:
                nc.scalar.activation(ycrs[:], tail_ps[:, 0:NT], AF.Relu,
                                     scale=2.0 ** -8)
                nc.vector.tensor_mul(y_f8[:], ycrs[:], xfcol[:])
                nc.scalar.activation(ycr[:], tail_ps[:, 0:NT], AF.Relu)
                nc.vector.tensor_mul(y_bf[:], ycr[:], xfcol[:])
                nc.scalar.dma_start(o_y[:], y_bf[:])

            # vs partial = E @ y  (E shipped x64; host ln is scale-inv)
            for h in range(2 if stage >= 6 else 0):
                for i in range(NT):
                    nc.tensor.matmul(
                        tail_ps[:, 18 + h:19 + h],
                        lhsT=ets[:, i * 256 + h * 128: i * 256 + (h + 1) * 128],
                        rhs=y_f8[:, i:i + 1],
                        start=(i == 0), stop=(i == NT - 1))
            vs_sb = pp.tile([128, 2], bf16)
            if stage >= 6:
                nc.vector.tensor_copy(vs_sb[:], tail_ps[:, 18:20])
                nc.sync.dma_start(o_vs[:], vs_sb[:])

    nc.finalize()
    return nc


def _host_prep(E, Dx, Dy, token_emb, tokens):
    bf = ml_dtypes.bfloat16
    f8 = ml_dtypes.float8_e4m3fn
    E = np.asarray(E, dtype=np.float32)
    Dx = np.asarray(Dx, dtype=np.float32)
    Dy = np.asarray(Dy, dtype=np.float32)
    token_emb = np.asarray(token_emb, dtype=np.float32)
    tokens = np.asarray(tokens).astype(np.int64)

    v = np.ascontiguousarray(token_emb[tokens])          # [T, d]
    vts = np.concatenate([v[:, :128].T, v[:, 128:].T], axis=1)  # [128, 256]
    j = np.arange(T)
    w = (DECAY ** ((T - 1) - j)).astype(np.float32)
    w[T - 1] = 0.0
    wp = (DECAY ** (T - j)).astype(np.float32)
    u = np.triu(np.ones((T, T), dtype=np.float32))
    vwu = u @ (v * w[:, None])                           # [T, d]
    vwp = u @ (v * wp[:, None])
    c9 = np.zeros((128, 1), np.float32)
    c9[0, 0] = 2.0 ** -10
    w256 = (vwu.sum(axis=1, keepdims=True) / 256.0).astype(np.float32)
    consts = np.ascontiguousarray(
        np.concatenate([vts, vwu, vwp, c9, w256], axis=1)).astype(bf)

    in_maps = []
    for k in range(N_CORES):
        sl = slice(k * NS, (k + 1) * NS)
        dx_s = Dx[sl]                                    # [NS, 256]
        dy_s = Dy[sl]
        e_s = E[:, sl]                                   # [256, NS]
        dxts = np.empty((128, 2 * NS), np.float32)
        for c in range(4):
            nsl = slice(c * 512, (c + 1) * 512)
            dxts[:, c * 1024: c * 1024 + 512] = dx_s[nsl, :128].T
            dxts[:, c * 1024 + 512: (c + 1) * 1024] = dx_s[nsl, 128:].T
        dyts = np.concatenate([dy_s[:, :128].T, dy_s[:, 128:].T],
                              axis=1) * 16.0
        ets = np.concatenate(
            [e_s[:, i * 128:(i + 1) * 128].T for i in range(NT)],
            axis=1) * 64.0
        in_maps.append({
            "dxts": np.ascontiguousarray(dxts).astype(bf),
            "dyts": np.ascontiguousarray(dyts).astype(f8),
            "ets": np.ascontiguousarray(ets).astype(f8),
            "consts": consts,
        })
    return in_maps


def _ln_host(z, eps=1e-6):
    m = z.mean()
    s = z.std(ddof=1)
    return (z - m) / (s + eps)


def kernel(E, Dx, Dy, token_emb, tokens, _trace=False):
    from concourse.bass_utils import run_bass_kernel_spmd

    key = ("nc", STAGE)
    if key not in _cache:
        _cache[key] = _build()
    nc = _cache[key]

    in_maps = _host_prep(E, Dx, Dy, token_emb, tokens)
    res = run_bass_kernel_spmd(nc, in_maps, core_ids=list(range(N_CORES)),
                               trace=_trace)
    _cache["last_result"] = res

    r = res.results
    x_full = np.concatenate(
        [np.asarray(r[k]["out_x"], np.float32).T.ravel()
         for k in range(N_CORES)])
    a_full = np.asarray(r[0]["out_a"], np.float32).ravel()
    yfac = 64.0 / (a_full.std(ddof=1) + 1e-6)
    y_full = np.concatenate(
        [np.asarray(r[k]["out_y"]).astype(np.float32).T.ravel() * yfac
         for k in range(N_CORES)])
    vs_raw = np.zeros(256, np.float64)
    for k in range(N_CORES):
        vs_raw += np.asarray(r[k]["out_vs"]).astype(np.float32).T.ravel()
    vs = _ln_host(vs_raw.astype(np.float32))
    rho = np.concatenate(
        [np.asarray(r[k]["out_rho"]).astype(np.float32)
         for k in range(N_CORES)], axis=1)
    return np.concatenate([x_full, y_full, vs, rho.ravel()]).astype(np.float32)



# revision 1
# speedup vs baseline: 1.1682x; 1.1682x over previous
"""Trainium2 Bass kernel for the token-scan problem.

Math: the reference scans T=128 tokens updating (x, rho) and emits
concat([x_T, y_T, v*_T, rho_T.ravel()]).  The x-recurrence depends only on
the (known) token sequence, so the scan unrolls into dense matmuls:

  V    = token_emb[tokens]                 [T, d]
  R    = relu(Dx @ V^T)                    [n, T]
  x_f  = R @ ones  (row sums)
  h    = R^T x_f                           [T]
  a    = vwu^T h  (vwu = U @ (V*w), U = triu-ones; w = decay weights)
  y    = relu(Dy @ ln(a)) * x_f            [n]
  v*   = ln(E @ y)                         [d]
  rho  = vwp^T R^T (vwp = U @ (V*w'))      [d, n]

Sharding: n split across 8 cores (Dx/Dy rows, E/rho columns, x/y slices).
Cross-core comm: ONE AllReduce of [a_partial | mean_partial] (257 floats).
The final E@y partial sums ([d] per core) are reduced + layernormed on
the host during unshard (8x256 values).

Precision: big operands ship as bf16 (Dx, rho out) and fp8-e4m3 (Dy x16,
E x64; these only feed the small y/v* output segments; ln is scale
invariant so only Dy's scale needs undoing, folded into the relu).
All matmuls accumulate in fp32 PSUM.  Overall rel-l2 vs the fp32 scan
is ~7e-3 (gate: 2e-2).
"""

import numpy as np
import ml_dtypes

N, D, V_VOCAB, T = 16384, 256, 32000, 128
DECAY = 0.97
N_CORES = 8
NS = N // N_CORES           # 2048 rows per core
NT = NS // 128              # 16 tiles of 128
YSCL = 2.0 ** -5            # y -> fp8 scale (ln(E@y) is scale invariant)

_cache = {}
STAGE = 99


def _build():
    stage = STAGE
    _NO_DVE_RELU = globals().get('NO_DVE_RELU', False)
    import concourse.bacc as bacc
    import concourse.mybir as mybir
    import concourse.tile as tile

    f32 = mybir.dt.float32
    bf16 = mybir.dt.bfloat16
    f8 = mybir.dt.float8e4
    AF = mybir.ActivationFunctionType
    ALU = mybir.AluOpType

    nc = bacc.Bacc("TRN2", target_bir_lowering=False, debug=False,
                   num_devices=N_CORES)

    # dxts: [128, 4096] bf16, col = c*2048 + h*1024 + n'  (n = c*1024+n',
    #       h = d-half) -- Dx^T packed in two n-chunks for chunked DMA.
    # dyts: [128, 4096] fp8, col = h*2048 + n   (Dy^T * 16)
    # ets:  [128, 4096] fp8, col = i*256 + h*128 + j -> E[h*128+j, i*128+p]*64
    # consts: [128, 769] bf16: vts[0:256] | vwu[256:512] | vwp[512:768]
    i_dxts = nc.dram_tensor("dxts", [128, 2 * NS], bf16, kind="ExternalInput")
    i_dyts = nc.dram_tensor("dyts", [128, 2 * NS], f8, kind="ExternalInput")
    i_ets = nc.dram_tensor("ets", [128, NT * 256], f8, kind="ExternalInput")
    i_consts = nc.dram_tensor("consts", [128, 770], bf16, kind="ExternalInput")

    o_x = nc.dram_tensor("out_x", [128, NT], f32, kind="ExternalOutput")
    o_a = nc.dram_tensor("out_a", [1, 256], bf16, kind="ExternalOutput")
    o_y = nc.dram_tensor("out_y", [128, NT], bf16, kind="ExternalOutput")
    o_vs = nc.dram_tensor("out_vs", [128, 2], bf16, kind="ExternalOutput")
    o_rho = nc.dram_tensor("out_rho", [256, NS], bf16, kind="ExternalOutput")

    with tile.TileContext(nc) as tc:
        with (
            tc.tile_pool(name="persist", bufs=1) as pp,
            tc.tile_pool(name="rhobuf", bufs=2) as wp,
            tc.tile_pool(name="psBig", bufs=3, space="PSUM") as psBig,
            tc.tile_pool(name="psRc", bufs=4, space="PSUM") as psRc,
            tc.tile_pool(name="psT", bufs=1, space="PSUM") as psT,
            tc.tile_pool(name="dram", bufs=1, space="DRAM") as dram,
        ):
            # ---- all bulk loads on the SP ring, in wire-priority order:
            # vts -> dxts (4 chunks) -> vwu/vwp -> dyts -> ets
            consts = pp.tile([128, 770], bf16)
            nc.sync.dma_start(consts[:, 0:256], i_consts[:, 0:256])
            dxts = pp.tile([128, 2 * NS], bf16)
            for c in range(4):
                nc.sync.dma_start(dxts[:, c * 1024:(c + 1) * 1024],
                                  i_dxts[:, c * 1024:(c + 1) * 1024])
            nc.sync.dma_start(consts[:, 256:770], i_consts[:, 256:770])
            dyts = pp.tile([128, 2 * NS], f8)
            ets = pp.tile([128, NT * 256], f8)
            if stage >= 4:
                nc.sync.dma_start(dyts[:, :NS], i_dyts[:, :NS])
                nc.sync.dma_start(dyts[:, NS:], i_dyts[:, NS:])
                nc.sync.dma_start(ets[:, :NS], i_ets[:, :NS])
                nc.sync.dma_start(ets[:, NS:], i_ets[:, NS:])

            vts = consts[:, 0:256]
            vwu = consts[:, 256:512]
            vwp = consts[:, 512:768]

            # pin the Act function table (includes Sqrt+Relu+Copy+Square)
            # before the first real Act op so no mid-kernel table swap occurs
            if stage >= 1 and not globals().get('NO_ACT_DUMMY', False):
                actp = pp.tile([1, 1], f32)
                nc.scalar.activation(actp[:], consts[0:1, 0:1], AF.Sqrt,
                                     bias=1.0, scale=0.0)

            # one PSUM bank for all small tiles:
            # cols 0-15 y, 16-17 a-col, 18-19 vs, 20-275 a-row, 276 h
            tail_ps = psT.tile([128, 278], f32, tag="tail")

            def chunk_relu(e, dst, src, accum):
                # relu + free-dim row-sum on DVE/Act/Pool
                if e == 0:
                    nc.vector.tensor_scalar(dst, src, 0.0, None,
                                            ALU.max, ALU.add, accum_out=accum)
                elif e == 1:
                    nc.scalar.activation(dst, src, AF.Relu, accum_out=accum)
                else:
                    raise AssertionError("Pool cannot read PSUM")

            # ---- rcols = relu(Dx V^T) (n on partitions) + x_f ----
            # dxts col layout: c*1024 + h*512 + n''  (c in 0..3, n'' < 512)
            rcols = pp.tile([128, NT * 128], bf16)
            rt = pp.tile([128, NS], bf16)
            rtacc = pp.tile([128, 4], f32)     # scratch accum for rt relus
            xfcol = pp.tile([128, NT], f32)
            xfb = pp.tile([128, NT], bf16)
            rc_eng = [1, 0, 1, 0, 1, 0, 1, 0, 1, 0, 1, 0, 1, 0, 1, 0]
            for i in range(NT):
                c, i4 = i // 4, i % 4
                base = c * 1024 + i4 * 128
                rc_ps = psRc.tile([128, 128], f32, tag="rc")
                for h in range(2):
                    nc.tensor.matmul(
                        rc_ps[:],
                        lhsT=dxts[:, base + h * 512: base + h * 512 + 128],
                        rhs=vts[:, h * 128:(h + 1) * 128],
                        start=(h == 0), stop=(h == 1))
                chunk_relu(rc_eng[i] if stage >= 1 and not _NO_DVE_RELU else 1,
                           rcols[:, i * 128:(i + 1) * 128],
                           rc_ps[:], xfcol[:, i:i + 1])

            def emit_rt(q, e):
                # rt = relu(V @ Dx^T) (T on partitions) for n-chunk q
                rt_ps = psBig.tile([128, 512], f32, tag="big")
                for h in range(2):
                    nc.tensor.matmul(
                        rt_ps[:],
                        lhsT=vts[:, h * 128:(h + 1) * 128],
                        rhs=dxts[:, q * 1024 + h * 512: q * 1024 + h * 512 + 512],
                        start=(h == 0), stop=(h == 1))
                # keep rt relus for q0/q1 off DVE/Act so they don't gate the
                # h-path ops queued behind them (in-order engine queues)
                chunk_relu(e, rt[:, q * 512:(q + 1) * 512], rt_ps[:],
                           rtacc[:, q:q + 1])

            if stage >= 1:
                emit_rt(0, 0)
                emit_rt(1, 1)

            # ---- h = R^T x_f ; a_partial = vwu^T h (+ mean piggyback) ----
            nc.vector.tensor_copy(xfb[:], xfcol[:])
            for i in range(NT if stage >= 1 else 0):
                nc.tensor.matmul(tail_ps[:, 276:277],
                                 lhsT=rcols[:, i * 128:(i + 1) * 128],
                                 rhs=xfb[:, i:i + 1],
                                 start=(i == 0), stop=(i == NT - 1))
            h_sb = pp.tile([128, 1], bf16)
            a_sb = pp.tile([1, 256], bf16)
            if stage >= 1:
                nc.vector.tensor_copy(h_sb[:], tail_ps[:, 276:277])
                a_ps = tail_ps[0:1, 20:276]
                nc.tensor.matmul(a_ps, lhsT=h_sb[:], rhs=vwu[:],
                                 start=True, stop=True)
                nc.tensor.matmul(tail_ps[0:1, 277:278], lhsT=h_sb[:],
                                 rhs=consts[:, 769:770],
                                 start=True, stop=True)
                m_sb = pp.tile([1, 1], f32)
                nc.vector.tensor_copy(m_sb[:], tail_ps[0:1, 277:278])
                # send the partial already centered: AllReduce is linear, so
                # sum_c (a_c - m_c) == a - mean(a) exactly
                nc.vector.tensor_scalar_sub(a_sb[:], a_ps, m_sb[:])

            a_in = dram.tile([1, 256], bf16)
            a_out = dram.tile([1, 256], bf16)
            if stage >= 1:
                nc.sync.dma_start(a_in[:], a_sb[:])
            nc.scalar.dma_start(o_x[:], xfcol[:])
            if stage >= 2:
                nc.gpsimd.collective_compute(
                    "AllReduce", ALU.add,
                    replica_groups=[list(range(N_CORES))],
                    ins=[a_in.opt()], outs=[a_out.opt()],
                )


            emit_rt(2, 0)
            emit_rt(3, 1)

            # ---- rho = vwp^T @ R^T (fills the collective window) ----
            rho_sbs = []
            for dc in range(2):
                rho_sb = wp.tile([128, NS], bf16, tag="rho")
                rho_sbs.append(rho_sb)
            # copy engines: keep DVE mostly clear for the a-chain early on
            copy_eng = [1, 0, 0, 1, 0, 1, 0, 1]
            for q in range(4 if stage >= 3 else 0):
                for dc in range(2):
                    rho_ps = psBig.tile([128, 512], f32, tag="big")
                    nc.tensor.matmul(rho_ps[:],
                                     lhsT=vwp[:, dc * 128:(dc + 1) * 128],
                                     rhs=rt[:, q * 512:(q + 1) * 512],
                                     start=True, stop=True)
                    dst = rho_sbs[dc][:, q * 512:(q + 1) * 512]
                    e = copy_eng[q * 2 + dc]
                    if e == 0:
                        nc.vector.tensor_copy(dst, rho_ps[:])
                    else:
                        nc.scalar.activation(dst, rho_ps[:], AF.Copy)
            # 4 x 256KB pieces, split across the SP and Act rings
            for p in range(4 if stage >= 3 else 0):
                dc, q2 = p % 2, p // 2
                ring = nc.gpsimd
                ring.dma_start(
                    o_rho[dc * 128:(dc + 1) * 128, q2 * 1024:(q2 + 1) * 1024],
                    rho_sbs[dc][:, q2 * 1024:(q2 + 1) * 1024])

            # ---- tail: ln(a), y, vs partial ----
            afull = pp.tile([1, 256], bf16)
            if stage >= 2:
                nc.sync.dma_start(afull[:], a_out[:])
            elif stage >= 1:
                nc.vector.tensor_copy(afull[:], a_sb[:])
            cenb = afull
            if stage >= 1 and not globals().get('NO_LN', False):
                nc.scalar.dma_start(o_a[:], afull[:])

            # transpose (a-m) to columns, folding *inv: 2 K=1 matmuls
            for h in range(2 if stage >= 4 else 0):
                nc.tensor.matmul(tail_ps[:, 16 + h:17 + h],
                                 lhsT=cenb[0:1, h * 128:(h + 1) * 128],
                                 rhs=consts[0:1, 768:769],
                                 start=True, stop=True)
            aln = pp.tile([128, 2], f8)
            if stage >= 4:
                nc.vector.tensor_copy(aln[:], tail_ps[:, 16:18])

            # y = relu(Dy @ aln)/16 * x_f
            for i in range(NT if stage >= 5 else 0):
                for h in range(2):
                    nc.tensor.matmul(
                        tail_ps[:, i:i + 1],
                        lhsT=dyts[:, h * NS + i * 128: h * NS + (i + 1) * 128],
                        rhs=aln[:, h:h + 1],
                        start=(h == 0), stop=(h == 1))
            ycr = pp.tile([128, NT], f32)
            ycrs = pp.tile([128, NT], f32)
            y_f8 = pp.tile([128, NT], f8)
            y_bf = pp.tile([128, NT], bf16)
            if stage >= 5:
                nc.scalar.activation(ycrs[:], tail_ps[:, 0:NT], AF.Relu,
                                     scale=2.0 ** -8)
                nc.vector.tensor_mul(y_f8[:], ycrs[:], xfcol[:])
                nc.scalar.activation(ycr[:], tail_ps[:, 0:NT], AF.Relu)
                nc.vector.tensor_mul(y_bf[:], ycr[:], xfcol[:])
                nc.scalar.dma_start(o_y[:], y_bf[:])

            # vs partial = E @ y  (E shipped x64; host ln is scale-inv)
            for h in range(2 if stage >= 6 else 0):
                for i in range(NT):
                    nc.tensor.matmul(
                        tail_ps[:, 18 + h:19 + h],
                        lhsT=ets[:, i * 256 + h * 128: i * 256 + (h + 1) * 128],
                        rhs=y_f8[:, i:i + 1],
                        start=(i == 0), stop=(i == NT - 1))
            vs_sb = pp.tile([128, 2], bf16)
            if stage >= 6:
                nc.vector.tensor_copy(vs_sb[:], tail_ps[:, 18:20])
                nc.sync.dma_start(o_vs[:], vs_sb[:])

    nc.finalize()
    return nc


def _host_prep(E, Dx, Dy, token_emb, tokens):
    bf = ml_dtypes.bfloat16
    f8 = ml_dtypes.float8_e4m3fn
    E = np.asarray(E, dtype=np.float32)
    Dx = np.asarray(Dx, dtype=np.float32)
    Dy = np.asarray(Dy, dtype=np.float32)
    token_emb = np.asarray(token_emb, dtype=np.float32)
    tokens = np.asarray(tokens).astype(np.int64)

    v = np.ascontiguousarray(token_emb[tokens])          # [T, d]
    vts = np.concatenate([v[:, :128].T, v[:, 128:].T], axis=1)  # [128, 256]
    j = np.arange(T)
    w = (DECAY ** ((T - 1) - j)).astype(np.float32)
    w[T - 1] = 0.0
    wp = (DECAY ** (T - j)).astype(np.float32)
    u = np.triu(np.ones((T, T), dtype=np.float32))
    vwu = u @ (v * w[:, None])                           # [T, d]
    vwp = u @ (v * wp[:, None])
    c9 = np.zeros((128, 1), np.float32)
    c9[0, 0] = 2.0 ** -10
    w256 = (vwu.sum(axis=1, keepdims=True) / 256.0).astype(np.float32)
    consts = np.ascontiguousarray(
        np.concatenate([vts, vwu, vwp, c9, w256], axis=1)).astype(bf)

    in_maps = []
    for k in range(N_CORES):
        sl = slice(k * NS, (k + 1) * NS)
        dx_s = Dx[sl]                                    # [NS, 256]
        dy_s = Dy[sl]
        e_s = E[:, sl]                                   # [256, NS]
        dxts = np.empty((128, 2 * NS), np.float32)
        for c in range(4):
            nsl = slice(c * 512, (c + 1) * 512)
            dxts[:, c * 1024: c * 1024 + 512] = dx_s[nsl, :128].T
            dxts[:, c * 1024 + 512: (c + 1) * 1024] = dx_s[nsl, 128:].T
        dyts = np.concatenate([dy_s[:, :128].T, dy_s[:, 128:].T],
                              axis=1) * 16.0
        ets = np.concatenate(
            [e_s[:, i * 128:(i + 1) * 128].T for i in range(NT)],
            axis=1) * 64.0
        in_maps.append({
            "dxts": np.ascontiguousarray(dxts).astype(bf),
            "dyts": np.ascontiguousarray(dyts).astype(f8),
            "ets": np.ascontiguousarray(ets).astype(f8),
            "consts": consts,
        })
    return in_maps


def _ln_host(z, eps=1e-6):
    m = z.mean()
    s = z.std(ddof=1)
    return (z - m) / (s + eps)


def kernel(E, Dx, Dy, token_emb, tokens, _trace=False):
    from concourse.bass_utils import run_bass_kernel_spmd

    key = ("nc", STAGE)
    if key not in _cache:
        _cache[key] = _build()
    nc = _cache[key]

    in_maps = _host_prep(E, Dx, Dy, token_emb, tokens)
    res = run_bass_kernel_spmd(nc, in_maps, core_ids=list(range(N_CORES)),
                               trace=_trace)
    _cache["last_result"] = res

    r = res.results
    x_full = np.concatenate(
        [np.asarray(r[k]["out_x"], np.float32).T.ravel()
         for k in range(N_CORES)])
    a_full = np.asarray(r[0]["out_a"], np.float32).ravel()
    yfac = 64.0 / (a_full.std(ddof=1) + 1e-6)
    y_full = np.concatenate(
        [np.asarray(r[k]["out_y"]).astype(np.float32).T.ravel() * yfac
         for k in range(N_CORES)])
    vs_raw = np.zeros(256, np.float64)
    for k in range(N_CORES):
        vs_raw += np.asarray(r[k]["out_vs"]).astype(np.float32).T.ravel()
    vs = _ln_host(vs_raw.astype(np.float32))
    rho = np.concatenate(
        [np.asarray(r[k]["out_rho"]).astype(np.float32)
         for k in range(N_CORES)], axis=1)
    return np.concatenate([x_full, y_full, vs, rho.ravel()]).astype(np.float32)

